# revision 8
# baseline (speedup 1.0000x reference)
"""CLAHE/LCN kernel for Trainium2, 8-core data parallel.

Math (per image, 31x31 'same' zero-padded box window):
    S  = box2d(x)   (sum)      Q = box2d(x^2)   (sum)
    mean = S/961, var = Q/961 - mean^2, std = sqrt(var)
    out  = 0.2*x + 0.4 + 0.4*tanh(0.25*(x-mean)/std)

I/O strategy: x and x^2 are shipped to the device as bf16 (host converts),
the device returns th = tanh(0.25*norm) as bf16, and the host applies the
final affine y = 0.2*x_f32 + 0.4*th + 0.4 (exact x, one fused numpy pass).
This halves HBM traffic vs f32 in / f32 out and removes the x^2 square and
final blend from the engine budget.

Box filter on PE: image block X_b (rows 128b..128b+127) as stationary lhsT
[K=128, M=128] against a banded 0/1 moving operand computes the column
31-box of X transposed; two such fused transpose+box stages give the full
2D box in natural layout.

Pipeline: phases p=0..IMGS; phase p interleaves stage-1 of image p with
stage-2 of image p-1 at w-tile granularity, so the PE stream round-robins
four PSUM rings (s1 ring of 2 [128,1024] tiles = 4 banks, S ring 1 = 2,
Q ring 1 = 2; 8 banks total) and rarely stalls on evacuation.

Engine split per image (elementwise is the bottleneck; PSUM is only
reachable from DVE/ACT — GpSimd has no PSUM port):
  ACT : part of t1t evac, m = -2c*S (Copy w/ scale, f32), rcp = AbsRsqrt,
        th = Tanh (grouped after rcp per image to limit table loads)
  DVE : t1x evac, part of t1t evac, v4 = 4c*Q - m^2 (STT), most of
        num = x + 0.5*m (STT, all-SBUF -> 2x mode)
  Pool: A' = m*m (STT form), z = num*rcp (STT form), rest of num
"""

import threading

import numpy as np
import ml_dtypes

# ---------------------------------------------------------------- constants
B_FULL = 32          # full batch
NCORES = 8
IMGS = B_FULL // NCORES  # images per core
H = W = 1024
P = 128              # partitions
NBLK = H // P        # 8 row blocks per image
NQ = 4               # quarters per image (2 row-tiles each)
KWIN = 31
HALF = KWIN // 2     # 15
AREA_INV = 1.0 / (KWIN * KWIN)  # 1/961

# tuning knobs (engine assignment per image)
EVAC_T_ACT = 3       # of the 8 t1t evac chunks, how many go to ACT (rest DVE)
NUM_POOL = 2       # of the 4 num chunks, how many go to Pool (rest DVE)

_lock = threading.Lock()
_compiled = None  # (nc, band_np)


def _band_spec():
    """Per h-block b: (lo, hi, offset into packed band array)."""
    spec = []
    off = 0
    for b in range(NBLK):
        lo = max(0, P * b - HALF)
        hi = min(H, P * b + P + HALF + 1)  # 128b+143
        spec.append((lo, hi, off))
        off += hi - lo
    return spec, off


def _band_np():
    spec, total = _band_spec()
    band = np.zeros((P, total), np.float32)
    for b, (lo, hi, off) in enumerate(spec):
        for h in range(P):
            gh = P * b + h
            r0 = max(lo, gh - HALF)
            r1 = min(hi, gh + HALF + 1)
            band[h, off + (r0 - lo): off + (r1 - lo)] = 1.0
    return band.astype(ml_dtypes.bfloat16)


def _mm_segments():
    """Matmul segment list for one output tile [128, 1024] in a single
    2-bank PSUM tile: (b, seg0, seg1, band_off, start, stop), segments
    clipped to PSUM bank boundaries (512 fp32); start=True on the first MM
    touching each bank, stop=True on the last."""
    spec, _ = _band_spec()
    per_bank = {0: [], 1: []}
    for b, (lo, hi, off) in enumerate(spec):
        for bank in (0, 1):
            s0 = max(lo, 512 * bank)
            s1 = min(hi, 512 * bank + 512)
            if s1 > s0:
                per_bank[bank].append((b, s0, s1, off + (s0 - lo)))
    out = []
    for bank in (0, 1):
        segs = per_bank[bank]
        for i, (b, s0, s1, boff) in enumerate(segs):
            out.append((b, s0, s1, boff, i == 0, i == len(segs) - 1))
    return out


def _patch_act_tables():
    """Hollow every table set except the two this kernel uses, so the
    selector maps Copy/Abs_reciprocal_sqrt to one set and Tanh to the
    other (2 loads per image instead of per-quarter thrash). Dict order
    (set IDs) is unchanged so the emitted IDs stay valid."""
    import concourse.bacc as bacc_mod
    if getattr(bacc_mod, "_clahe_tables_patched", False):
        return
    orig = bacc_mod.get_activation_tables
    keep = {"abs_reciprocal_sqrt_and_small", "silu_and_others"}

    def patched(arch):
        tabs = dict(orig(arch))
        for k in tabs:
            if k not in keep:
                tabs[k] = set()
        return tabs

    bacc_mod.get_activation_tables = patched
    bacc_mod._clahe_tables_patched = True


def _build():
    import concourse.bacc as bacc
    import concourse.tile as tile
    from concourse import mybir
    from concourse.tile import add_dep_helper

    _patch_act_tables()

    f32 = mybir.dt.float32
    bf16 = mybir.dt.bfloat16
    ALU = mybir.AluOpType
    ACT = mybir.ActivationFunctionType

    spec, band_w = _band_spec()
    mm_segs = _mm_segments()
    c = AREA_INV

    nc = bacc.Bacc("TRN2", target_bir_lowering=False, debug=False,
                   num_devices=NCORES)
    x_ext = nc.dram_tensor("x", [IMGS * H, W], bf16, kind="ExternalInput")
    x2_ext = nc.dram_tensor("x2", [IMGS * H, W], bf16, kind="ExternalInput")
    band_ext = nc.dram_tensor("band", [P, band_w], bf16, kind="ExternalInput")
    y_ext = nc.dram_tensor("y", [IMGS * H, W], bf16, kind="ExternalOutput")
    x_ap = x_ext.ap()
    x2_ap = x2_ext.ap()
    y_ap = y_ext.ap()

    with tile.TileContext(nc) as tc:
        from contextlib import ExitStack
        with ExitStack() as ctx:
            def pool(name, bufs, space="SBUF"):
                return ctx.enter_context(
                    tc.tile_pool(name=name, bufs=bufs, space=space))

            singles = pool("singles", 1)
            p_x = pool("p_x", 2)       # x full image [P,8,W] bf16
            p_x2 = pool("p_x2", 2)     # x^2 full image [P,8,W] bf16
            p_t1 = pool("p_t1", 2)     # t1x/t1t [P,8,W] bf16 (2 tags)
            p_m = pool("p_m", 2)       # m = -2c*S per quarter [P,2,W] f32
            p_a = pool("p_a", 2)       # A' = m^2 per (q,j) [P,W] f32
            p_v = pool("p_v", 2)       # v4 per quarter [P,2,W] bf16
            p_rcp = pool("p_rcp", 2)   # rcp per quarter [P,2,W] bf16
            p_num = pool("p_num", 2)   # num per quarter [P,2,W] bf16
            p_z = pool("p_z", 2)       # z per quarter [P,2,W] bf16
            p_th = pool("p_th", 2)     # th per quarter [P,2,W] bf16
            ps_1 = pool("ps1", 2, space="PSUM")   # stage-1 [P,1024] ring 2
            ps_s = pool("psS", 1, space="PSUM")   # stage-2 S [P,1024] ring 1
            ps_q = pool("psQ", 1, space="PSUM")   # stage-2 Q [P,1024] ring 1

            band_sb = singles.tile([P, band_w], bf16)
            nc.sync.dma_start(out=band_sb[:], in_=band_ext.ap())

            def stage_mms(ps, stat_slicer):
                """Banded MM group for one [128,1024] output tile into a
                2-bank PSUM tile `ps` (bank-clipped segments)."""
                for (b, s0, s1, boff, first, last) in mm_segs:
                    nc.tensor.matmul(
                        ps[:, s0:s1],
                        stat_slicer(b),
                        band_sb[:, boff: boff + (s1 - s0)],
                        start=first, stop=last,
                    )

            x_hist = {}
            t1_hist = {}
            state = {}  # per-image stage-2 tail state

            def load_img(p):
                xt = p_x.tile([P, NBLK, W], bf16, tag="x")
                x2t = p_x2.tile([P, NBLK, W], bf16, tag="x2")
                nc.sync.dma_start(out=xt[:], in_=img_rows(x_ap, p * H))
                nc.sync.dma_start(out=x2t[:], in_=img_rows(x2_ap, p * H))
                x_hist[p] = (xt, x2t)

            def s1_step(p, wt):
                xt, x2t = x_hist[p]
                t1x, t1t = t1_hist[p]
                ps = ps_1.tile([P, 1024], f32, tag="ps1")
                stage_mms(ps, lambda b: xt[:, b, wt * P:(wt + 1) * P])
                nc.vector.tensor_copy(t1x[:, wt, :], ps[:])
                ps = ps_1.tile([P, 1024], f32, tag="ps1")
                stage_mms(ps, lambda b: x2t[:, b, wt * P:(wt + 1) * P])
                if wt < EVAC_T_ACT:
                    nc.scalar.copy(out=t1t[:, wt, :], in_=ps[:])
                else:
                    nc.vector.tensor_copy(t1t[:, wt, :], ps[:])

            def s2_step(p, step):
                """Stage-2 unit (q=step//2, j=step%2) of image p, plus the
                per-quarter tail after odd steps."""
                t1x, t1t = t1_hist[p]
                st = state.setdefault(p, {})
                q, j = step // 2, step % 2
                if j == 0:
                    st['m'] = p_m.tile([P, 2, W], f32, tag="m", name="m")
                    st['v'] = p_v.tile([P, 2, W], bf16, tag="v4", name="v4")
                mq, vq = st['m'], st['v']
                mm = step
                ps_S = ps_s.tile([P, 1024], f32, tag="psS")
                stage_mms(ps_S, lambda b: t1x[:, b, mm * P:(mm + 1) * P])
                ps_Q = ps_q.tile([P, 1024], f32, tag="psQ")
                stage_mms(ps_Q, lambda b: t1t[:, b, mm * P:(mm + 1) * P])
                # m = -c*S = -mean   (f32: mean^2 needs full precision)
                nc.scalar.activation(mq[:, j], ps_S[:], ACT.Copy,
                                     bias=0.0, scale=-c)
                # A' = m*m = mean^2   (DVE TT, all-SBUF f32 -> 2x mode)
                aq = p_a.tile([P, W], f32, tag="A")
                nc.vector.tensor_mul(aq[:], mq[:, j], mq[:, j])
                # v = c*Q - A' = var   (bf16)
                nc.vector.scalar_tensor_tensor(
                    vq[:, j], ps_Q[:], c, aq[:],
                    op0=ALU.mult, op1=ALU.subtract)
                if j == 0:
                    return
                # ---- per-quarter tail ----
                xt, _ = x_hist[p]
                # rcp = 1/sqrt(16*v) = 1/(4*std)   (bf16)
                rq = p_rcp.tile([P, 2, W], bf16, tag="rcp")
                rcp_i = nc.scalar.activation(rq[:], vq[:],
                                             ACT.Abs_reciprocal_sqrt,
                                             bias=0.0, scale=16.0)
                st.setdefault('rcps', []).append(rcp_i)
                # num = x + m = x - mean   (bf16 out)
                nq_t = p_num.tile([P, 2, W], bf16, tag="num")
                eng = nc.gpsimd if q < NUM_POOL else nc.vector
                eng.tensor_add(nq_t[:], xt[:, 2 * q:2 * q + 2, :], mq[:])
                # z = num * rcp = 0.25*norm   (Pool TT)
                zt = p_z.tile([P, 2, W], bf16, tag="z")
                nc.gpsimd.tensor_mul(zt[:], nq_t[:], rq[:])
                st.setdefault('zs', []).append(zt)
                if q == NQ - 1:
                    # th = tanh(z), grouped after the image's last rcp so
                    # the ACT table set switches only twice per image.
                    # Last image: let tanh interleave (fills the drain).
                    for qq in range(NQ):
                        tht = p_th.tile([P, 2, W], bf16, tag="th")
                        th_i = nc.scalar.activation(
                            tht[:], st['zs'][qq][:], ACT.Tanh,
                            bias=0.0, scale=1.0)
                        if p < IMGS - 1:
                            add_dep_helper(th_i.ins, st['rcps'][-1].ins,
                                           reason="batch ACT table sets")
                        nc.sync.dma_start(
                            out=y_rows(y_ap, p * H + 256 * qq), in_=tht[:])
                    del state[p]

            # ---- phase loop ----
            load_img(0)
            for p in range(IMGS + 1):
                if p + 1 < IMGS:
                    load_img(p + 1)
                if p < IMGS:
                    t1_hist[p] = (
                        p_t1.tile([P, NBLK, W], bf16, tag="t1x", name="t1x"),
                        p_t1.tile([P, NBLK, W], bf16, tag="t1t", name="t1t"),
                    )
                for step in range(NBLK):
                    if p < IMGS:
                        s1_step(p, step)
                    if p >= 1:
                        s2_step(p - 1, step)
                if p >= 1:
                    t1_hist.pop(p - 1)
                    x_hist.pop(p - 1, None)

    nc.compile()
    return nc


def img_rows(dram_ap, row0):
    """DRAM AP view [P, 8, W]: element (p, t, c) <-> dram[row0+128t+p, c]."""
    sl = dram_ap[row0: row0 + H, :]
    return sl.rearrange("(t p) c -> p t c", p=P)


def y_rows(dram_ap, row0):
    """DRAM AP view [P, 2, W]: element (p, t, c) <-> dram[row0+128t+p, c]."""
    sl = dram_ap[row0: row0 + 256, :]
    return sl.rearrange("(t p) c -> p t c", p=P)


def _get_compiled():
    global _compiled
    with _lock:
        if _compiled is None:
            band = np.ascontiguousarray(_band_np())
            nc = _build()
            _compiled = (nc, band)
    return _compiled


def _run(x, trace=False, **kw):
    from concourse.bass_utils import run_bass_kernel_spmd

    bf16 = ml_dtypes.bfloat16
    nc, band = _get_compiled()
    x = np.asarray(x, dtype=np.float32).reshape(B_FULL, H, W)
    xb16 = x.astype(bf16)
    xbf = xb16.astype(np.float32)
    x2b16 = (xbf * xbf).astype(bf16)
    core_ids = list(range(NCORES))
    in_maps = []
    for i in core_ids:
        sl = slice(IMGS * i, IMGS * (i + 1))
        in_maps.append({
            "x": np.ascontiguousarray(xb16[sl].reshape(IMGS * H, W)),
            "x2": np.ascontiguousarray(x2b16[sl].reshape(IMGS * H, W)),
            "band": band,
        })
    res = run_bass_kernel_spmd(nc, in_maps, core_ids, trace=trace, **kw)
    th = np.concatenate(
        [np.asarray(res.results[i]["y"], dtype=np.float32)
         .reshape(IMGS, H, W) for i in core_ids], axis=0)
    out = (0.2 * x + 0.4 * th + 0.4).astype(np.float32)
    return out.reshape(B_FULL, 1, H, W), res


def kernel(x):
    out, _ = _run(x, trace=False)
    return out


# revision 19
# speedup vs baseline: 1.0837x; 1.0837x over previous
"""CLAHE/LCN kernel for Trainium2, 8-core data parallel.

Math (per image, 31x31 'same' zero-padded box window):
    S  = box2d(x)   (sum)      Q = box2d(x^2)   (sum)
    mean = S/961, var = Q/961 - mean^2, std = sqrt(var)
    out  = 0.2*x + 0.4 + 0.4*tanh(0.25*(x-mean)/std)

I/O strategy: x and x^2 are shipped to the device as bf16 (host converts),
the device returns th = tanh(0.25*norm) as bf16, and the host applies the
final affine y = 0.2*x_f32 + 0.4*th + 0.4 (exact x, one fused numpy pass).
This halves HBM traffic vs f32 in / f32 out and removes the x^2 square and
final blend from the engine budget.

Box filter on PE: image block X_b (rows 128b..128b+127) as stationary lhsT
[K=128, M=128] against a banded 0/1 moving operand computes the column
31-box of X transposed; two such fused transpose+box stages give the full
2D box in natural layout.

Pipeline: phases p=0..IMGS; phase p interleaves stage-1 of image p with
stage-2 of image p-1 at w-tile granularity, so the PE stream round-robins
four PSUM rings (s1 ring of 2 [128,1024] tiles = 4 banks, S ring 1 = 2,
Q ring 1 = 2; 8 banks total) and rarely stalls on evacuation.

Engine split per image (elementwise is the bottleneck; PSUM is only
reachable from DVE/ACT — GpSimd has no PSUM port):
  ACT : part of t1t evac, m = -2c*S (Copy w/ scale, f32), rcp = AbsRsqrt,
        th = Tanh (grouped after rcp per image to limit table loads)
  DVE : t1x evac, part of t1t evac, v4 = 4c*Q - m^2 (STT), most of
        num = x + 0.5*m (STT, all-SBUF -> 2x mode)
  Pool: A' = m*m (STT form), z = num*rcp (STT form), rest of num
"""

import threading

import numpy as np
import ml_dtypes

# ---------------------------------------------------------------- constants
B_FULL = 32          # full batch
NCORES = 8
IMGS = B_FULL // NCORES  # images per core
H = W = 1024
P = 128              # partitions
NBLK = H // P        # 8 row blocks per image
NQ = 4               # quarters per image (2 row-tiles each)
KWIN = 31
HALF = KWIN // 2     # 15
AREA_INV = 1.0 / (KWIN * KWIN)  # 1/961

# tuning knobs (engine assignment per image)
EVAC_T_ACT = 2       # of the 8 t1t evac chunks, how many go to ACT (rest DVE)

_lock = threading.Lock()
_compiled = None  # (nc, band_np)


def _band_spec():
    """Per h-block b: (lo, hi, offset into packed band array)."""
    spec = []
    off = 0
    for b in range(NBLK):
        lo = max(0, P * b - HALF)
        hi = min(H, P * b + P + HALF + 1)  # 128b+143
        spec.append((lo, hi, off))
        off += hi - lo
    return spec, off


def _band_np():
    spec, total = _band_spec()
    band = np.zeros((P, total), np.float32)
    for b, (lo, hi, off) in enumerate(spec):
        for h in range(P):
            gh = P * b + h
            r0 = max(lo, gh - HALF)
            r1 = min(hi, gh + HALF + 1)
            band[h, off + (r0 - lo): off + (r1 - lo)] = 1.0
    return band.astype(ml_dtypes.bfloat16)


def _mm_segments():
    """Matmul segment list for one output tile [128, 1024] in a single
    2-bank PSUM tile: (b, seg0, seg1, band_off, start, stop), segments
    clipped to PSUM bank boundaries (512 fp32); start=True on the first MM
    touching each bank, stop=True on the last. Ordered b-major so the two
    bank-halves of a boundary-crossing block are adjacent (same stationary
    back to back -> walrus can reuse the loaded weights)."""
    spec, _ = _band_spec()
    raw = []
    for b, (lo, hi, off) in enumerate(spec):
        for bank in (0, 1):
            s0 = max(lo, 512 * bank)
            s1 = min(hi, 512 * bank + 512)
            if s1 > s0:
                raw.append((b, s0, s1, off + (s0 - lo)))
    first_of_bank = {}
    last_of_bank = {}
    for i, (b, s0, s1, boff) in enumerate(raw):
        bank = 0 if s0 < 512 else 1
        first_of_bank.setdefault(bank, i)
        last_of_bank[bank] = i
    out = []
    for i, (b, s0, s1, boff) in enumerate(raw):
        bank = 0 if s0 < 512 else 1
        out.append((b, s0, s1, boff,
                    first_of_bank[bank] == i, last_of_bank[bank] == i))
    return out


def _register_var_op():
    """Runtime-register a custom DVE op: out = in0*s0 - (in1*s1)^2.
    Computes var directly from the S and Q PSUM fields in one DVE pass,
    replacing the mean evacuation + square + subtract chain (and keeping
    mean^2 at f32 internally, which the cancellation needs)."""
    import concourse.dve_ops as dve_ops
    from concourse.dve_spec import Spec, Src0, Src1, C0, C1, lower, sq
    from concourse.dve_spec import _has_src1
    from concourse.dve_uop import DveOpSpec

    name = "VAR_CLAHE"
    for op in dve_ops.OPS:
        if op.name == name:
            return op
    spec = Spec(
        body=(Src0 * C0) - sq(Src1 * C1),
        reference=lambda in0, in1, s0, s1, imm2: (
            in0.astype(np.float32) * s0
            - (in1.astype(np.float32) * s1) ** 2),
    )
    row = dve_ops._CUSTOM_DVE_ROW_BASE + len(dve_ops.OPS)
    shas = {}
    for ver in ("v3",):
        uops = lower(spec, ver=ver)
        shas[ver] = DveOpSpec(name=name, opcode=row, uops=uops,
                              rd1_en=_has_src1(spec)).sha(ver)
    op = dve_ops.DveOp(name, spec, subdim=False, uops_sha=shas)
    dve_ops.OPS.append(op)
    dve_ops._SUB_OPCODE_FOR_NAME[name] = row
    dve_ops.CUSTOM_DVE_SPECS[name] = op.spec
    return op


def _patch_act_tables():
    """Hollow every table set except the two this kernel uses, so the
    selector maps Copy/Abs_reciprocal_sqrt to one set and Tanh to the
    other (2 loads per image instead of per-quarter thrash). Dict order
    (set IDs) is unchanged so the emitted IDs stay valid."""
    import concourse.bacc as bacc_mod
    if getattr(bacc_mod, "_clahe_tables_patched", False):
        return
    orig = bacc_mod.get_activation_tables
    keep = {"abs_reciprocal_sqrt_and_small", "silu_and_others"}

    def patched(arch):
        tabs = dict(orig(arch))
        for k in tabs:
            if k not in keep:
                tabs[k] = set()
        return tabs

    bacc_mod.get_activation_tables = patched
    bacc_mod._clahe_tables_patched = True


def _build():
    import concourse.bacc as bacc
    import concourse.tile as tile
    from concourse import mybir
    from concourse.tile import add_dep_helper

    _patch_act_tables()
    var_op = _register_var_op()

    f32 = mybir.dt.float32
    bf16 = mybir.dt.bfloat16
    ALU = mybir.AluOpType
    ACT = mybir.ActivationFunctionType

    spec, band_w = _band_spec()
    mm_segs = _mm_segments()
    c = AREA_INV

    nc = bacc.Bacc("TRN2", target_bir_lowering=False, debug=False,
                   num_devices=NCORES)
    x_ext = nc.dram_tensor("x", [IMGS * H, W], bf16, kind="ExternalInput")
    x2_ext = nc.dram_tensor("x2", [IMGS * H, W], bf16, kind="ExternalInput")
    band_ext = nc.dram_tensor("band", [P, band_w], bf16, kind="ExternalInput")
    y_ext = nc.dram_tensor("y", [IMGS * H, W], bf16, kind="ExternalOutput")
    x_ap = x_ext.ap()
    x2_ap = x2_ext.ap()
    y_ap = y_ext.ap()

    with tile.TileContext(nc) as tc:
        from contextlib import ExitStack
        with ExitStack() as ctx:
            def pool(name, bufs, space="SBUF"):
                return ctx.enter_context(
                    tc.tile_pool(name=name, bufs=bufs, space=space))

            singles = pool("singles", 1)
            p_x = pool("p_x", 2)       # x full image [P,8,W] bf16
            p_x2 = pool("p_x2", 2)     # x^2 full image [P,8,W] bf16
            p_t1 = pool("p_t1", 2)     # t1x/t1t [P,8,W] bf16 (2 tags)
            p_m = pool("p_m", 2)       # m = -mean per quarter [P,2,W] f32
            p_v = pool("p_v", 2)       # v16 per quarter [P,2,W] bf16
            p_rcp = pool("p_rcp", 2)   # rcp per quarter [P,2,W] bf16
            p_num = pool("p_num", 2)   # num per quarter [P,2,W] bf16
            p_z = pool("p_z", 2)       # z per quarter [P,2,W] bf16
            p_th = pool("p_th", 2)     # th per quarter [P,2,W] bf16
            ps_1 = pool("ps1", 2, space="PSUM")   # stage-1 [P,1024] ring 2
            ps_s = pool("psS", 1, space="PSUM")   # stage-2 S [P,1024] ring 1
            ps_q = pool("psQ", 1, space="PSUM")   # stage-2 Q [P,1024] ring 1

            band_sb = singles.tile([P, band_w], bf16)
            nc.sync.dma_start(out=band_sb[:], in_=band_ext.ap())

            def stage_mms(ps, stat_slicer):
                """Banded MM group for one [128,1024] output tile into a
                2-bank PSUM tile `ps` (bank-clipped segments)."""
                for (b, s0, s1, boff, first, last) in mm_segs:
                    nc.tensor.matmul(
                        ps[:, s0:s1],
                        stat_slicer(b),
                        band_sb[:, boff: boff + (s1 - s0)],
                        start=first, stop=last,
                    )

            x_hist = {}
            t1_hist = {}
            state = {}  # per-image stage-2 tail state

            def load_img(p):
                xt = p_x.tile([P, NBLK, W], bf16, tag="x")
                x2t = p_x2.tile([P, NBLK, W], bf16, tag="x2")
                nc.sync.dma_start(out=xt[:], in_=img_rows(x_ap, p * H))
                nc.sync.dma_start(out=x2t[:], in_=img_rows(x2_ap, p * H))
                x_hist[p] = (xt, x2t)

            def s1_step(p, wt):
                xt, x2t = x_hist[p]
                t1x, t1t = t1_hist[p]
                ps = ps_1.tile([P, 1024], f32, tag="ps1")
                stage_mms(ps, lambda b: xt[:, b, wt * P:(wt + 1) * P])
                nc.vector.tensor_copy(t1x[:, wt, :], ps[:])
                ps = ps_1.tile([P, 1024], f32, tag="ps1")
                stage_mms(ps, lambda b: x2t[:, b, wt * P:(wt + 1) * P])
                if wt < EVAC_T_ACT:
                    nc.scalar.copy(out=t1t[:, wt, :], in_=ps[:])
                else:
                    nc.vector.tensor_copy(t1t[:, wt, :], ps[:])

            def s2_step(p, step):
                """Stage-2 unit (q=step//2, j=step%2) of image p, plus the
                per-quarter tail after odd steps."""
                t1x, t1t = t1_hist[p]
                st = state.setdefault(p, {})
                q, j = step // 2, step % 2
                if j == 0:
                    st['m'] = p_m.tile([P, 2, W], f32, tag="m", name="m")
                    st['v'] = p_v.tile([P, 2, W], bf16, tag="v16", name="v16")
                    st['num'] = p_num.tile([P, 2, W], bf16, tag="num",
                                           name="num")
                mq, vq, nq_t = st['m'], st['v'], st['num']
                xt, _ = x_hist[p]
                mm = step
                ps_S = ps_s.tile([P, 1024], f32, tag="psS")
                stage_mms(ps_S, lambda b: t1x[:, b, mm * P:(mm + 1) * P])
                ps_Q = ps_q.tile([P, 1024], f32, tag="psQ")
                stage_mms(ps_Q, lambda b: t1t[:, b, mm * P:(mm + 1) * P])
                # m = -c*S = -mean   (f32: mean^2 needs full precision;
                # DVE may read only one PSUM operand per instruction, so
                # S must land in SBUF for the fused var op)
                nc.scalar.activation(mq[:, j], ps_S[:], ACT.Copy,
                                     bias=0.0, scale=-c)
                # v16 = 16c*Q - (4*m)^2 = 16*var   (one fused DVE pass,
                # mean^2 kept at f32 inside the op)
                nc.vector._custom_dve(
                    var_op, out=vq[:, j], in0=ps_Q[:], in1=mq[:, j],
                    s0=16.0 * c, s1=4.0)
                # num = x + m = x - mean   (bf16, all-SBUF)
                nc.vector.scalar_tensor_tensor(
                    nq_t[:, j], mq[:, j], 1.0, xt[:, mm, :],
                    op0=ALU.mult, op1=ALU.add)
                if j == 0:
                    return
                # ---- per-quarter tail ----
                # rcp = 1/sqrt(16*var) = 1/(4*std)   (bf16)
                rq = p_rcp.tile([P, 2, W], bf16, tag="rcp")
                rcp_i = nc.scalar.activation(rq[:], vq[:],
                                             ACT.Abs_reciprocal_sqrt,
                                             bias=0.0, scale=1.0)
                st.setdefault('rcps', []).append(rcp_i)
                # z = num * rcp = 0.25*norm   (Pool TT)
                zt = p_z.tile([P, 2, W], bf16, tag="z")
                nc.gpsimd.tensor_mul(zt[:], nq_t[:], rq[:])
                st.setdefault('zs', []).append(zt)
                if q == NQ - 1:
                    # th = tanh(z), grouped after the image's last rcp so
                    # the ACT table set switches only twice per image.
                    # Last image: let tanh interleave (fills the drain).
                    for qq in range(NQ):
                        tht = p_th.tile([P, 2, W], bf16, tag="th")
                        th_i = nc.scalar.activation(
                            tht[:], st['zs'][qq][:], ACT.Tanh,
                            bias=0.0, scale=1.0)
                        if p < IMGS - 1:
                            add_dep_helper(th_i.ins, st['rcps'][-1].ins,
                                           reason="batch ACT table sets")
                        nc.sync.dma_start(
                            out=y_rows(y_ap, p * H + 256 * qq), in_=tht[:])
                    del state[p]

            # ---- phase loop ----
            load_img(0)
            for p in range(IMGS + 1):
                if p + 1 < IMGS:
                    load_img(p + 1)
                if p < IMGS:
                    t1_hist[p] = (
                        p_t1.tile([P, NBLK, W], bf16, tag="t1x", name="t1x"),
                        p_t1.tile([P, NBLK, W], bf16, tag="t1t", name="t1t"),
                    )
                for step in range(NBLK):
                    if p < IMGS:
                        s1_step(p, step)
                    if p >= 1:
                        s2_step(p - 1, step)
                if p >= 1:
                    t1_hist.pop(p - 1)
                    x_hist.pop(p - 1, None)

    nc.compile()
    return nc


def img_rows(dram_ap, row0):
    """DRAM AP view [P, 8, W]: element (p, t, c) <-> dram[row0+128t+p, c]."""
    sl = dram_ap[row0: row0 + H, :]
    return sl.rearrange("(t p) c -> p t c", p=P)


def y_rows(dram_ap, row0):
    """DRAM AP view [P, 2, W]: element (p, t, c) <-> dram[row0+128t+p, c]."""
    sl = dram_ap[row0: row0 + 256, :]
    return sl.rearrange("(t p) c -> p t c", p=P)


def _get_compiled():
    global _compiled
    with _lock:
        if _compiled is None:
            band = np.ascontiguousarray(_band_np())
            nc = _build()
            _compiled = (nc, band)
    return _compiled


def _run(x, trace=False, **kw):
    from concourse.bass_utils import run_bass_kernel_spmd

    bf16 = ml_dtypes.bfloat16
    nc, band = _get_compiled()
    x = np.asarray(x, dtype=np.float32).reshape(B_FULL, H, W)
    xb16 = x.astype(bf16)
    xbf = xb16.astype(np.float32)
    x2b16 = (xbf * xbf).astype(bf16)
    core_ids = list(range(NCORES))
    in_maps = []
    for i in core_ids:
        sl = slice(IMGS * i, IMGS * (i + 1))
        in_maps.append({
            "x": np.ascontiguousarray(xb16[sl].reshape(IMGS * H, W)),
            "x2": np.ascontiguousarray(x2b16[sl].reshape(IMGS * H, W)),
            "band": band,
        })
    res = run_bass_kernel_spmd(nc, in_maps, core_ids, trace=trace, **kw)
    th = np.concatenate(
        [np.asarray(res.results[i]["y"], dtype=np.float32)
         .reshape(IMGS, H, W) for i in core_ids], axis=0)
    out = (0.2 * x + 0.4 * th + 0.4).astype(np.float32)
    return out.reshape(B_FULL, 1, H, W), res


def kernel(x):
    out, _ = _run(x, trace=False)
    return out
